# revision 2
# baseline (speedup 1.0000x reference)
"""NeighborAttention (B=4, N=4096, K=32, C=128, H=4) on 8 Trainium2 cores.

Data-parallel: the flattened (B*N) node axis is sharded across 8 cores;
the four small weight matrices are replicated. Inside each core everything
is channel-major ([row 4d+h, node-major free]):

  ET = (E*mask)^T            -> K,V of masked neighbors are exactly 0
  KT = WK' @ ET, VT = WV' @ ET, QT = (WQ'/sqrt(d)) @ XT        (PE)
  prod = KT * broadcast_j(QT)                                  (DVE)
  s_rep = Hrep @ prod        head-summed scores, replicated over d (PE)
  e = exp(s_rep)             no max-subtraction needed: |s| is small, and
                             softmax is shift-invariant               (ACT)
  z = sum_j e - (K - cnt[n]) masked j contribute exp(0)=1; host sends the
                             per-node count correction               (DVE)
  uv = e * VT;  umax = max_j uv;  usum = sum_j uv                    (DVE)
  out = (WO_mean+WO_sum)' @ (usum/z) + WO_max' @ (umax/z)            (PE)

attn sums to exactly 1, so aggr_mean == aggr_sum (within 1e-8) and the
mean/sum W_O blocks fold together on the host.
"""
import numpy as np
import concourse.bass as bass
import concourse.bacc as bacc
import concourse.mybir as mybir
from concourse import tile
from concourse.bass_utils import run_bass_kernel_spmd

F32 = mybir.dt.float32
AX = mybir.AxisListType.X
ALU = mybir.AluOpType

K = 32
C = 128
H = 4
D = 32
NCORES = 8

SUB_N = 16
SUB_COLS = SUB_N * K
CHUNK_N = 256
CHUNK_COLS = CHUNK_N * K

_NC_CACHE = {}


def _build_nc(nloc):
    assert nloc % CHUNK_N == 0
    if nloc in _NC_CACHE:
        return _NC_CACHE[nloc]
    nchunks = nloc // CHUNK_N
    nsub = CHUNK_COLS // SUB_COLS

    nc = bacc.Bacc()
    et = nc.dram_tensor("et", [C, nloc * K], F32, kind="ExternalInput")
    xt = nc.dram_tensor("xt", [C, nloc], F32, kind="ExternalInput")
    wqt = nc.dram_tensor("wqt", [C, C], F32, kind="ExternalInput")
    wkt = nc.dram_tensor("wkt", [C, C], F32, kind="ExternalInput")
    wvt = nc.dram_tensor("wvt", [C, C], F32, kind="ExternalInput")
    hrep = nc.dram_tensor("hrep", [C, C], F32, kind="ExternalInput")
    wost = nc.dram_tensor("wost", [C, C], F32, kind="ExternalInput")
    wo3t = nc.dram_tensor("wo3t", [C, C], F32, kind="ExternalInput")
    mcorr = nc.dram_tensor("mcorr", [C, nloc], F32, kind="ExternalInput")
    out = nc.dram_tensor("out", [C, nloc], F32, kind="ExternalOutput")

    with tile.TileContext(nc) as tc:
        with tc.tile_pool(name="wts", bufs=1) as wpool, \
             tc.tile_pool(name="xin", bufs=1) as xpool, \
             tc.tile_pool(name="etp", bufs=3) as etpool, \
             tc.tile_pool(name="work", bufs=5) as work, \
             tc.tile_pool(name="acc", bufs=2) as accp, \
             tc.tile_pool(name="outp", bufs=1) as outp, \
             tc.tile_pool(name="pkv", bufs=4, space="PSUM") as pkv, \
             tc.tile_pool(name="psc", bufs=2, space="PSUM") as psc, \
             tc.tile_pool(name="psmall", bufs=1, space="PSUM") as psmall:

            w_q = wpool.tile([C, C], F32, tag="wq")
            w_k = wpool.tile([C, C], F32, tag="wk")
            w_v = wpool.tile([C, C], F32, tag="wv")
            w_h = wpool.tile([C, C], F32, tag="wh")
            w_os = wpool.tile([C, C], F32, tag="wos")
            w_o3 = wpool.tile([C, C], F32, tag="wo3")
            nc.sync.dma_start(w_q[:], wqt[:])
            nc.sync.dma_start(w_k[:], wkt[:])
            nc.sync.dma_start(w_v[:], wvt[:])
            nc.sync.dma_start(w_h[:], hrep[:])
            nc.sync.dma_start(w_os[:], wost[:])
            nc.sync.dma_start(w_o3[:], wo3t[:])

            xt_sb = xpool.tile([C, nloc], F32, tag="xt")
            nc.sync.dma_start(xt_sb[:], xt[:])
            mc_sb = xpool.tile([C, nloc], F32, tag="mc")
            nc.sync.dma_start(mc_sb[:], mcorr[:])

            out_sb = outp.tile([C, nloc], F32, tag="osb")

            for ch in range(nchunks):
                n0 = ch * CHUNK_N
                c0 = ch * CHUNK_COLS

                et_sb = etpool.tile([C, CHUNK_COLS], F32, tag="et")
                nc.sync.dma_start(et_sb[:], et[:, c0:c0 + CHUNK_COLS])

                q_ps = psmall.tile([C, CHUNK_N], F32, tag="qps")
                nc.tensor.matmul(q_ps[:], w_q[:], xt_sb[:, n0:n0 + CHUNK_N],
                                 start=True, stop=True)
                q_sb = work.tile([C, CHUNK_N], F32, tag="qsb")
                nc.vector.tensor_copy(q_sb[:], q_ps[:])

                umax_c = accp.tile([C, CHUNK_N], F32, tag="umax")
                usum_c = accp.tile([C, CHUNK_N], F32, tag="usum")
                z_c = accp.tile([C, CHUNK_N], F32, tag="zc")

                for s in range(nsub):
                    sc0 = s * SUB_COLS
                    sn0 = s * SUB_N
                    esl = et_sb[:, sc0:sc0 + SUB_COLS]

                    kt_ps = pkv.tile([C, SUB_COLS], F32, tag="kv")
                    nc.tensor.matmul(kt_ps[:], w_k[:], esl, start=True, stop=True)
                    vt_ps = pkv.tile([C, SUB_COLS], F32, tag="kv")
                    nc.tensor.matmul(vt_ps[:], w_v[:], esl, start=True, stop=True)

                    qb = q_sb[:, sn0:sn0 + SUB_N].unsqueeze(2).broadcast_to(
                        (C, SUB_N, K))
                    prod = work.tile([C, SUB_COLS], F32, tag="prod")
                    nc.vector.tensor_mul(
                        prod[:].rearrange("p (n j) -> p n j", j=K),
                        kt_ps[:].rearrange("p (n j) -> p n j", j=K),
                        qb)

                    s_ps = psc.tile([C, SUB_COLS], F32, tag="srep")
                    nc.tensor.matmul(s_ps[:], w_h[:], prod[:],
                                     start=True, stop=True)

                    erep = work.tile([C, SUB_COLS], F32, tag="erep")
                    nc.scalar.activation(erep[:], s_ps[:],
                                         mybir.ActivationFunctionType.Exp)

                    uv = work.tile([C, SUB_COLS], F32, tag="uv")
                    nc.vector.tensor_mul(uv[:], erep[:], vt_ps[:])

                    uv_v = uv[:].rearrange("p (n j) -> p n j", j=K)
                    e_v = erep[:].rearrange("p (n j) -> p n j", j=K)
                    nc.vector.tensor_reduce(
                        umax_c[:, sn0:sn0 + SUB_N], uv_v, axis=AX, op=ALU.max)
                    nc.vector.tensor_reduce(
                        usum_c[:, sn0:sn0 + SUB_N], uv_v, axis=AX, op=ALU.add)
                    nc.vector.tensor_reduce(
                        z_c[:, sn0:sn0 + SUB_N], e_v, axis=AX, op=ALU.add)

                zcor = work.tile([C, CHUNK_N], F32, tag="zcor")
                nc.vector.tensor_sub(zcor[:], z_c[:], mc_sb[:, n0:n0 + CHUNK_N])
                # fully-masked nodes: umax/usum rows are exactly 0, so any
                # finite 1/z gives the correct 0 output — just avoid inf*0.
                nc.vector.tensor_scalar_max(zcor[:], zcor[:], 1e-20)
                rz = work.tile([C, CHUNK_N], F32, tag="rz")
                nc.vector.reciprocal(rz[:], zcor[:])

                wsn = work.tile([C, CHUNK_N], F32, tag="wsn")
                nc.vector.tensor_mul(wsn[:], usum_c[:], rz[:])
                mxn = work.tile([C, CHUNK_N], F32, tag="mxn")
                nc.vector.tensor_mul(mxn[:], umax_c[:], rz[:])

                o_ps = psmall.tile([C, CHUNK_N], F32, tag="ops")
                nc.tensor.matmul(o_ps[:], w_os[:], wsn[:], start=True, stop=False)
                nc.tensor.matmul(o_ps[:], w_o3[:], mxn[:], start=False, stop=True)
                nc.scalar.copy(out_sb[:, n0:n0 + CHUNK_N], o_ps[:])

            nc.sync.dma_start(out[:], out_sb[:])

    nc.compile()
    _NC_CACHE[nloc] = nc
    return nc


def _perm_dh(w):
    """[(h*32+d), cin] -> [cin, (4d+h)]"""
    wt = np.asarray(w).reshape(H, D, -1)
    return np.ascontiguousarray(np.transpose(wt, (2, 1, 0)).reshape(-1, H * D))


def kernel(h_X, h_E, mask_attn, W_Q, W_K, W_V, W_O):
    h_X = np.asarray(h_X, dtype=np.float32)
    h_E = np.asarray(h_E, dtype=np.float32)
    mask_attn = np.asarray(mask_attn)
    W_Q = np.asarray(W_Q, dtype=np.float32)
    W_K = np.asarray(W_K, dtype=np.float32)
    W_V = np.asarray(W_V, dtype=np.float32)
    W_O = np.asarray(W_O, dtype=np.float32)

    B, N, Kn, Cin = h_E.shape
    BN = B * N
    nloc = BN // NCORES

    maskf = mask_attn.astype(np.float32)
    e_m = (h_E * maskf[..., None]).reshape(BN, Kn, Cin)
    xf = h_X.reshape(BN, -1)
    cnt = maskf.reshape(BN, Kn).sum(axis=1)

    wqt = _perm_dh(W_Q / np.sqrt(D))
    wkt = _perm_dh(W_K)
    wvt = _perm_dh(W_V)

    idx = np.arange(C)
    hh = idx % H
    hrep = (hh[:, None] == hh[None, :]).astype(np.float32)

    wos = W_O[:, :C] + W_O[:, C:2 * C]
    wo3 = W_O[:, 2 * C:]
    wost = np.ascontiguousarray(
        wos.T.reshape(H, D, C).transpose(1, 0, 2).reshape(C, C))
    wo3t = np.ascontiguousarray(
        wo3.T.reshape(H, D, C).transpose(1, 0, 2).reshape(C, C))

    in_maps = []
    for i in range(NCORES):
        sl = slice(i * nloc, (i + 1) * nloc)
        etc = np.ascontiguousarray(e_m[sl].reshape(nloc * Kn, Cin).T)
        xtc = np.ascontiguousarray(xf[sl].T)
        mc = np.ascontiguousarray(
            np.broadcast_to(Kn - cnt[sl], (C, nloc)).astype(np.float32))
        in_maps.append({
            "et": etc, "xt": xtc,
            "wqt": wqt, "wkt": wkt, "wvt": wvt, "hrep": hrep,
            "wost": wost, "wo3t": wo3t, "mcorr": mc,
        })

    nc = _build_nc(nloc)
    res = run_bass_kernel_spmd(nc, in_maps, core_ids=list(range(NCORES)))

    outf = np.empty((BN, C), np.float32)
    for i, r in enumerate(res.results):
        outf[i * nloc:(i + 1) * nloc] = r["out"].T
    return outf.reshape(B, N, C)
